# revision 1
# baseline (speedup 1.0000x reference)
"""Trainium2 Bass kernel for nn_AdaptiveGraphConv (gnn_message_passing).

Data-parallel over batch: B=64 split as 8 batch elements per NeuronCore,
params replicated. No collectives needed.

v2 "dup layout": per batch element, x (64, 7500) is stored column-split as
xdup (128, 3750) = [x[:, :3750] ; x[:, 3750:]].  This makes every small
matmul K=128 (full PE array) and every PSUM->SBUF copy full 128 partitions:

  1. cast-DMA x_b f32 -> bf16 into xdup halves
  2. P = (theta^T phi) @ X per half   -- paired matmuls K=64 rows 0:63 /
     64:127 (tile_position), one (128,500) PSUM tile per chunk
  3. M = sum_t x_t^T (theta^T phi) x_t via 30 K=128 matmuls into one
     (125,125) PSUM tile (5-t diagonal-block trick; both halves sum in)
     + 30 K=128 PE transposes xdup_chunk^T -> xt16 pairs
  4. diag blocks realigned via SBUF->SBUF DMA (engines can't address
     partition bases not 0 mod 32), softmax, adj = sum_k(A+B) + 3*C
  5. x_sum = blockdiag(adj) applied per chunk: 30 K=125 matmuls; outputs
     land as [h0;h1] pairs -> xsd (128, 3750)
  6. out = w1 @ x_sum + wr @ x as 2 accumulating K=128 matmuls per 500-col
     region (zero-padded weight halves) into single-bank PSUM tiles;
     BN+ReLU fused on ScalarE.  (Matmuls writing at a column offset into a
     multi-bank PSUM tile produce wrong results on HW -- keep tiles 1 bank.)
"""
import numpy as np
import ml_dtypes

B_, CIN, T_, N_ = 64, 64, 300, 25
COUT, EMB, KV = 128, 32, 3
EPS = 1e-5
NCORES = 8
BL = B_ // NCORES          # local batch per core
TN = T_ * N_               # 7500
HALF = TN // 2             # 3750, t-aligned (150*25) and 125-aligned
MC = 125
NMCH = HALF // MC          # 30 chunks of 125 cols per half

_CACHE = {}


def _build():
    import concourse.bacc as bacc
    import concourse.mybir as mybir
    from concourse import tile

    f32 = mybir.dt.float32
    bf16 = mybir.dt.bfloat16
    AF = mybir.ActivationFunctionType
    AX = mybir.AxisListType

    nc = bacc.Bacc("TRN2", target_bir_lowering=False, debug=False,
                   num_devices=NCORES)

    x = nc.dram_tensor("x", [BL, CIN, TN], f32, kind="ExternalInput")
    ksymT = nc.dram_tensor("ksymT", [CIN, CIN], bf16, kind="ExternalInput")
    w1t = nc.dram_tensor("w1t", [CIN, COUT], bf16, kind="ExternalInput")
    wrt = nc.dram_tensor("wrt", [CIN, COUT], bf16, kind="ExternalInput")
    ident = nc.dram_tensor("ident", [128, 128], bf16, kind="ExternalInput")
    asum = nc.dram_tensor("asum", [N_, N_], f32, kind="ExternalInput")
    sel = nc.dram_tensor("sel", [MC, N_], f32, kind="ExternalInput")
    selt = nc.dram_tensor("selt", [N_, MC], f32, kind="ExternalInput")
    maskf = nc.dram_tensor("maskf", [MC, MC], f32, kind="ExternalInput")
    bns = nc.dram_tensor("bns", [COUT, 1], f32, kind="ExternalInput")
    bnb = nc.dram_tensor("bnb", [COUT, 1], f32, kind="ExternalInput")
    out = nc.dram_tensor("out", [BL, COUT, TN], f32, kind="ExternalOutput")

    # 500-col regions per half (last is 250)
    REG = [(o, min(500, HALF - o)) for o in range(0, HALF, 500)]

    with tile.TileContext(nc) as tc:
        with (
            tc.tile_pool(name="const", bufs=1) as cpool,
            tc.tile_pool(name="xdup", bufs=3) as xpool,
            tc.tile_pool(name="pdup", bufs=2) as ppool,
            tc.tile_pool(name="xt16", bufs=3) as xtpool,
            tc.tile_pool(name="xsd", bufs=2) as xspool,
            tc.tile_pool(name="osb", bufs=6) as opool,
            tc.tile_pool(name="small", bufs=16) as spool,
            tc.tile_pool(name="bd", bufs=2) as bdpool,
            tc.tile_pool(name="pps", bufs=2, space="PSUM") as ppsum,
            tc.tile_pool(name="xsp", bufs=2, space="PSUM") as xspsum,
            tc.tile_pool(name="mps", bufs=1, space="PSUM") as mpsum,
            tc.tile_pool(name="ops", bufs=3, space="PSUM") as opsum,
        ):
            # constants: Ksym^T, w1^T, wr^T replicated on both partition
            # halves so K=64 matmuls can run from row groups 0-1 or 2-3
            ks2 = cpool.tile([128, CIN], bf16)
            nc.sync.dma_start(ks2[0:64, :], ksymT[:])
            nc.sync.dma_start(ks2[64:128, :], ksymT[:])
            # zero-padded weight variants: w1/wr live on the h-row half
            # only, so a K=128 matmul over the full [xsum;x]-dup tiles
            # contracts just that half (zero rows contribute nothing)
            wz = []
            for h in range(2):
                w1z = cpool.tile([128, COUT], bf16, tag=f"w1z{h}")
                nc.gpsimd.memset(w1z[:], 0.0)
                nc.sync.dma_start(w1z[64 * h:64 * h + 64, :], w1t[:])
                wrz = cpool.tile([128, COUT], bf16, tag=f"wrz{h}")
                nc.gpsimd.memset(wrz[:], 0.0)
                nc.sync.dma_start(wrz[64 * h:64 * h + 64, :], wrt[:])
                wz.append((w1z, wrz))
            idt = cpool.tile([128, 128], bf16)
            nc.sync.dma_start(idt[:], ident[:])
            as_t = cpool.tile([N_, N_], f32)
            nc.sync.dma_start(as_t[:], asum[:])
            sel_t = cpool.tile([MC, N_], f32)
            nc.sync.dma_start(sel_t[:], sel[:])
            selt_t = cpool.tile([N_, MC], f32)
            nc.sync.dma_start(selt_t[:], selt[:])
            mask_t = cpool.tile([MC, MC], f32)
            nc.sync.dma_start(mask_t[:], maskf[:])
            bns_t = cpool.tile([COUT, 1], f32)
            nc.sync.dma_start(bns_t[:], bns[:])
            bnb_t = cpool.tile([COUT, 1], f32)
            nc.sync.dma_start(bnb_t[:], bnb[:])

            def phase_a(b):
                """load + P + M/transposes + softmax -> bd for batch b."""
                xdup = xpool.tile([128, HALF], bf16)
                QB = [0, 1000, 2000, 3000, HALF]
                for q in range(4):
                    qs, qe = QB[q], QB[q + 1]
                    nc.gpsimd.dma_start(xdup[0:64, qs:qe], x[b, :, qs:qe])
                    nc.gpsimd.dma_start(xdup[64:128, qs:qe],
                                        x[b, :, HALF + qs:HALF + qe])

                pdup = ppool.tile([128, HALF], bf16)
                for o, w in REG:
                    pps = ppsum.tile([128, 500], f32, tag="pps")
                    nc.tensor.matmul(pps[0:64, 0:w], ks2[0:64, :],
                                     xdup[0:64, o:o + w],
                                     start=True, stop=True)
                    nc.tensor.matmul(pps[64:128, 0:w], ks2[64:128, :],
                                     xdup[64:128, o:o + w],
                                     start=True, stop=True,
                                     tile_position=(64, 64))
                    nc.vector.tensor_copy(pdup[:, o:o + w], pps[:, 0:w])

                mps = mpsum.tile([MC, MC], f32)
                xt16 = xtpool.tile([MC, NMCH * 128], bf16)
                for ci in range(NMCH):
                    sl = slice(ci * MC, (ci + 1) * MC)
                    nc.tensor.matmul(mps[:], xdup[:, sl], pdup[:, sl],
                                     start=(ci == 0),
                                     stop=(ci == NMCH - 1))
                for g in range(6):
                    xtp = xspsum.tile([MC, 5 * 128], bf16, tag="xsp")
                    for q in range(5):
                        ci = 5 * g + q
                        sl = slice(ci * MC, (ci + 1) * MC)
                        nc.tensor.transpose(xtp[:, q * 128:(q + 1) * 128],
                                            xdup[:, sl], idt[:])
                    nc.vector.tensor_copy(
                        xt16[:, g * 640:(g + 1) * 640], xtp[:])

                # diag-block realign on the PE: engines can't address
                # partition bases that aren't 0 mod 32, so compute
                # SEL^T @ (mps * blockmask) -> (25, 5*25) exactly in f32
                masked = spool.tile([MC, MC], f32, tag="masked")
                nc.vector.tensor_mul(masked[:], mps[:], mask_t[:])
                msp = mpsum.tile([N_, MC], f32, tag="mps")
                nc.tensor.matmul(msp[:], sel_t[:], masked[:],
                                 start=True, stop=True)
                msf = spool.tile([N_, MC], f32, tag="msf")
                nc.vector.tensor_copy(msf[:], msp[:])
                m01 = spool.tile([N_, N_], f32, tag="sm")
                nc.vector.tensor_add(m01[:], msf[:, 0:25], msf[:, 25:50])
                m23 = spool.tile([N_, N_], f32, tag="sm")
                nc.vector.tensor_add(m23[:], msf[:, 50:75], msf[:, 75:100])
                m03 = spool.tile([N_, N_], f32, tag="sm")
                nc.vector.tensor_add(m03[:], m01[:], m23[:])
                msum = spool.tile([N_, N_], f32, tag="sm")
                nc.vector.tensor_add(msum[:], m03[:], msf[:, 100:125])

                negmax = spool.tile([N_, 1], f32, tag="sv")
                nc.vector.reduce_max(negmax[:], msum[:], axis=AX.X,
                                     negate=True)
                expm = spool.tile([N_, N_], f32, tag="sm")
                ssum = spool.tile([N_, 1], f32, tag="sv")
                nc.scalar.activation(expm[:], msum[:], AF.Exp,
                                     bias=negmax[:], accum_out=ssum[:])
                rs = spool.tile([N_, 1], f32, tag="sv")
                nc.vector.reciprocal(rs[:], ssum[:])
                adjf = spool.tile([N_, N_], f32, tag="sm")
                nc.vector.tensor_scalar(adjf[:], expm[:], rs[:], float(KV),
                                        op0=mybir.AluOpType.mult,
                                        op1=mybir.AluOpType.mult)
                adjs = spool.tile([N_, N_], f32, tag="sm16")
                nc.vector.tensor_add(adjs[:], adjf[:], as_t[:])

                # bd = blockdiag(adj): broadcast adj down all 5 row-blocks
                # via one matmul, then mask columns block-wise
                bcast = mpsum.tile([MC, N_], f32, tag="mps")
                nc.tensor.matmul(bcast[:], selt_t[:], adjs[:],
                                 start=True, stop=True)
                bd = bdpool.tile([MC, MC], bf16)
                for t in range(5):
                    nc.vector.tensor_mul(
                        bd[:, t * 25:(t + 1) * 25], bcast[:],
                        mask_t[:, t * 25:(t + 1) * 25])
                return xdup, xt16, bd

            def phase_b(b, xdup, xt16, bd):
                """x_sum + output for batch b (consumes phase_a tiles)."""
                xsd = xspool.tile([128, HALF], bf16)
                for g in range(8):
                    base = 4 * g
                    cnt = min(4, NMCH - base)
                    xsp = xspsum.tile([128, 500], f32, tag="xsp")
                    for q in range(cnt):
                        ci = base + q
                        nc.tensor.matmul(xsp[:, q * MC:(q + 1) * MC],
                                         xt16[:, ci * 128:(ci + 1) * 128],
                                         bd[:], start=True, stop=True)
                    nc.vector.tensor_copy(
                        xsd[:, base * MC:(base + cnt) * MC],
                        xsp[:, 0:cnt * MC])

                for h in range(2):
                    w1z, wrz = wz[h]
                    for o, w in REG:
                        ops_t = opsum.tile([128, 500], f32)
                        nc.tensor.matmul(ops_t[:, 0:w], w1z[:],
                                         xsd[:, o:o + w],
                                         start=True, stop=False)
                        nc.tensor.matmul(ops_t[:, 0:w], wrz[:],
                                         xdup[:, o:o + w],
                                         start=False, stop=True)
                        osb = opool.tile([128, 500], f32)
                        nc.scalar.activation(osb[:, 0:w], ops_t[:, 0:w],
                                             AF.Relu, bias=bnb_t[:],
                                             scale=bns_t[:])
                        nc.sync.dma_start(
                            out[b, :, h * HALF + o:h * HALF + o + w],
                            osb[:, 0:w])

            # software pipeline: batch b's attention phase runs while batch
            # b-1's xsum/output phase waits on its softmax chain
            prev = None
            for b in range(BL):
                tiles = phase_a(b)
                if prev is not None:
                    phase_b(b - 1, *prev)
                prev = tiles
            phase_b(BL - 1, *prev)
    nc.finalize()
    return nc


def kernel(**inputs):
    x = np.ascontiguousarray(inputs["x"], dtype=np.float32)
    theta_w = inputs["theta_w"]
    phi_w = inputs["phi_w"]
    A, Bp = inputs["A"], inputs["Bparam"]
    w1, wr = inputs["w1"], inputs["wr"]
    b1, br = inputs["b1"], inputs["br"]
    gamma, beta = inputs["gamma"], inputs["beta"]
    rmean, rvar = inputs["rmean"], inputs["rvar"]

    bf = ml_dtypes.bfloat16
    ksymT = np.ascontiguousarray(phi_w.T @ theta_w).astype(bf)
    w1tv = np.ascontiguousarray(w1.T).astype(bf)
    wrtv = np.ascontiguousarray(wr.T).astype(bf)
    ident = np.eye(128, dtype=np.float32).astype(bf)
    asumv = np.ascontiguousarray((A + Bp).sum(0), dtype=np.float32)
    selv = np.ascontiguousarray(np.tile(np.eye(N_, dtype=np.float32), (5, 1)))
    seltv = np.ascontiguousarray(selv.T)
    maskv = np.zeros((MC, MC), np.float32)
    for t in range(5):
        maskv[t * N_:(t + 1) * N_, t * N_:(t + 1) * N_] = 1.0
    bnscale = (gamma / np.sqrt(rvar + EPS)).astype(np.float32)
    bnbias = ((b1 + br - rmean) * bnscale + beta).astype(np.float32)

    if "nc" not in _CACHE:
        _CACHE["nc"] = _build()
    nc = _CACHE["nc"]

    shared = {
        "ksymT": ksymT, "w1t": w1tv, "wrt": wrtv, "ident": ident,
        "asum": asumv, "sel": selv, "selt": seltv, "maskf": maskv,
        "bns": np.ascontiguousarray(bnscale[:, None]),
        "bnb": np.ascontiguousarray(bnbias[:, None]),
    }
    in_maps = []
    for i in range(NCORES):
        xi = np.ascontiguousarray(
            x[i * BL:(i + 1) * BL].reshape(BL, CIN, TN))
        in_maps.append({"x": xi, **shared})

    from concourse.bass_utils import run_bass_kernel_spmd
    res = run_bass_kernel_spmd(nc, in_maps, core_ids=list(range(NCORES)))
    outs = [np.asarray(r["out"], dtype=np.float32).reshape(BL, COUT, T_, N_)
            for r in res.results]
    return np.concatenate(outs, axis=0)

